# revision 1
# baseline (speedup 1.0000x reference)
"""Trainium2 Bass kernel for nn_Attention_65128884077225.

Math: the reference module broadcasts scores [B,H,S,1] along the softmax
axis, so every softmax row is constant -> attention weights are exactly
uniform (1/S). Hence z = mean_s(v) broadcast over s, and the whole module
collapses to, per batch b:

    c[b] = (mean_s x[b,s,:]) @ Wv @ Wout + (bv @ Wout + bout)
    out[b,s,:] = c[b]                      (constant across s)

where Wv = qkv_w[:, 2E:3E], bv = qkv_b[2E:3E].

Device kernel (SPMD on 8 cores, core c -> batch b=c//2, output half h=c%2):
  - read x[b] [2048,512] (16 tiles), tree-accumulate rows on DVE,
  - column-sum via 4 matmuls against a ones vector -> xsum^T [128,4],
  - 4-step accumulated matmul xsum @ Wc -> row [1,512] (+ folded bias),
  - outer-product broadcast row -> [128,512] tile,
  - write the tile 8x to cover the core's 1024 output rows.

Host only: fold Wc = (Wv @ Wout)/S and bc = bv @ Wout + bout (tiny 512^3
GEMM), shard inputs, and concatenate the per-core output halves.
"""

import os
import sys

import numpy as np

if "/opt/trn_rl_repo" not in sys.path and not any(
    p.endswith("trn_rl_repo") for p in sys.path
):
    sys.path.insert(0, "/opt/trn_rl_repo")

import concourse.bacc as bacc
import concourse.mybir as mybir
import concourse.tile as tile
from concourse.bass_utils import run_bass_kernel_spmd

B, S, E = 4, 2048, 512
N_CORES = 8
P = 128
N_XT = S // P          # 16 x-tiles of [128, 512]
HALF = S // 2          # 1024 rows of output per core
N_OT = HALF // P       # 8 output tiles
FP32 = mybir.dt.float32

_CACHE = {}


def build():
    """Build + compile the per-core Bass program (same for every core)."""
    if "nc" in _CACHE:
        return _CACHE["nc"]
    nc = bacc.Bacc(None, target_bir_lowering=False)
    x_d = nc.dram_tensor("x", [S, E], FP32, kind="ExternalInput")
    wc_d = nc.dram_tensor("wc", [E, E], FP32, kind="ExternalInput")
    bc_d = nc.dram_tensor("bc", [E], FP32, kind="ExternalInput")
    o_d = nc.dram_tensor("o", [HALF, E], FP32, kind="ExternalOutput")

    with tile.TileContext(nc) as tc:
        with (
            tc.tile_pool(name="xp", bufs=N_XT) as xp,
            tc.tile_pool(name="wp", bufs=4) as wp,
            tc.tile_pool(name="sp", bufs=1) as sp,
            tc.tile_pool(name="ps", bufs=1, space="PSUM") as ps,
        ):
            ones_col = sp.tile([P, 1], FP32, tag="ones_col")
            nc.vector.memset(ones_col[:], 1.0)
            ones_row = sp.tile([1, P], FP32, tag="ones_row")
            nc.vector.memset(ones_row[:], 1.0)

            bcr = sp.tile([1, E], FP32, tag="bcr")
            nc.sync.dma_start(bcr[:], bc_d[None, :])

            wts = []
            for k in range(4):
                wt = wp.tile([P, E], FP32, tag="w")
                nc.sync.dma_start(wt[:], wc_d[k * P : (k + 1) * P, :])
                wts.append(wt)

            xts = []
            for t in range(N_XT):
                xt = xp.tile([P, E], FP32, tag="x")
                nc.sync.dma_start(xt[:], x_d[t * P : (t + 1) * P, :])
                xts.append(xt)

            # tree-free serial accumulate: acc = sum_t x_t   [128, 512]
            acc = sp.tile([P, E], FP32, tag="acc")
            nc.vector.tensor_add(acc[:], xts[0][:], xts[1][:])
            for t in range(2, N_XT):
                nc.vector.tensor_add(acc[:], acc[:], xts[t][:])

            # column sums: xsum^T [128, 4]; xsumT[p, c] = sum_s x[s, c*128+p]
            p_red = ps.tile([P, 4], FP32, tag="red")
            for c in range(4):
                nc.tensor.matmul(
                    p_red[:, c : c + 1],
                    acc[:, c * P : (c + 1) * P],
                    ones_col[:],
                    start=True,
                    stop=True,
                )
            xsumT = sp.tile([P, 4], FP32, tag="xsumT")
            nc.vector.tensor_copy(xsumT[:], p_red[:])

            # c_row [1, 512] = xsum @ Wc  (accumulate over 4 k-chunks)
            p_crow = ps.tile([1, E], FP32, tag="crow")
            for k in range(4):
                nc.tensor.matmul(
                    p_crow[:],
                    xsumT[:, k : k + 1],
                    wts[k][:],
                    start=(k == 0),
                    stop=(k == 3),
                )
            crow = sp.tile([1, E], FP32, tag="crowsb")
            nc.vector.tensor_add(crow[:], p_crow[:], bcr[:])

            # broadcast row across 128 partitions via rank-1 matmul
            p_bc = ps.tile([P, E], FP32, tag="bc")
            nc.tensor.matmul(p_bc[:], ones_row[:], crow[:], start=True, stop=True)
            bcast = sp.tile([P, E], FP32, tag="bcast")
            nc.vector.tensor_copy(bcast[:], p_bc[:])

            for u in range(N_OT):
                nc.sync.dma_start(o_d[u * P : (u + 1) * P, :], bcast[:])

    nc.compile()
    _CACHE["nc"] = nc
    return nc


def _fold_weights(qkv_w, qkv_b, out_w, out_b):
    wv = np.asarray(qkv_w)[:, 2 * E : 3 * E].astype(np.float64)
    wc = (wv @ np.asarray(out_w).astype(np.float64) / S).astype(np.float32)
    bc = (
        np.asarray(qkv_b)[2 * E : 3 * E].astype(np.float64)
        @ np.asarray(out_w).astype(np.float64)
        + np.asarray(out_b)
    ).astype(np.float32)
    return np.ascontiguousarray(wc), np.ascontiguousarray(bc)


def _run(inputs, trace=False, **kwargs):
    nc = build()
    x = np.ascontiguousarray(np.asarray(inputs["x"], dtype=np.float32))
    wc, bc = _fold_weights(
        inputs["qkv_w"], inputs["qkv_b"], inputs["out_w"], inputs["out_b"]
    )
    in_maps = [{"x": x[c // 2], "wc": wc, "bc": bc} for c in range(N_CORES)]
    res = run_bass_kernel_spmd(
        nc, in_maps, core_ids=list(range(N_CORES)), trace=trace, **kwargs
    )
    out = np.empty((B, S, E), dtype=np.float32)
    for b in range(B):
        out[b, :HALF] = res.results[2 * b]["o"]
        out[b, HALF:] = res.results[2 * b + 1]["o"]
    return out, res


def kernel(**inputs) -> np.ndarray:
    out, _ = _run(inputs, trace=False)
    return out


# revision 2
# speedup vs baseline: 1.1849x; 1.1849x over previous
"""Trainium2 Bass kernel for nn_Attention_65128884077225.

Math: the reference module broadcasts scores [B,H,S,1] along the softmax
axis, so every softmax row is constant -> attention weights are exactly
uniform (1/S). Hence z = mean_s(v) broadcast over s, and the whole module
collapses to, per batch b:

    c[b] = (mean_s x[b,s,:]) @ Wv @ Wout + (bv @ Wout + bout)
    out[b,s,:] = c[b]                      (constant across s)

where Wv = qkv_w[:, 2E:3E], bv = qkv_b[2E:3E].

Sharding: 8 cores = 4 batches x 2 column-halves. Core c handles batch
b=c//2 and output columns [h*256, (h+1)*256), h=c%2. Each core reads the
full x[b] (needed for the mean), but only its half of the folded weight
matrix, and writes out[b][:, cols] (2 MiB).

Device kernel per core:
  - 16 DMA loads of x row-tiles [128,512], alternating across the two
    HWDGE rings (sync + scalar),
  - serial DVE add-chain accumulates the 16 tiles -> acc [128,512],
  - 4 matmuls vs a ones-vector give column sums xsum^T [128,4],
  - 4-step accumulated matmul xsum @ Wc_half -> row [1,256], + bias,
  - rank-1 matmul broadcasts the row -> [128,256] tile,
  - 16 stores of that tile cover out[b][:, cols] (written as a contiguous
    [2048,256] per-core output, reassembled on host).

Host only: fold Wc = (Wv @ Wout)/S and bc = bv @ Wout + bout (tiny host
GEMM), shard inputs, and concatenate the per-core outputs.
"""

import sys

import numpy as np

if "/opt/trn_rl_repo" not in sys.path and not any(
    p.endswith("trn_rl_repo") for p in sys.path
):
    sys.path.insert(0, "/opt/trn_rl_repo")

import concourse.bacc as bacc
import concourse.mybir as mybir
import concourse.tile as tile
from concourse.bass_utils import run_bass_kernel_spmd

B, S, E = 4, 2048, 512
N_CORES = 8
P = 128
N_XT = S // P          # 16 x-tiles of [128, 512]
EH = E // 2            # 256 output columns per core
N_OT = S // P          # 16 output tiles of [128, 256]
FP32 = mybir.dt.float32

_CACHE = {}


def build():
    """Build + compile the per-core Bass program (same for every core)."""
    if "nc" in _CACHE:
        return _CACHE["nc"]
    nc = bacc.Bacc(None, target_bir_lowering=False, enable_partition_id=False)
    x_d = nc.dram_tensor("x", [S, E], FP32, kind="ExternalInput")
    wc_d = nc.dram_tensor("wc", [E, EH], FP32, kind="ExternalInput")
    bc_d = nc.dram_tensor("bc", [EH], FP32, kind="ExternalInput")
    o_d = nc.dram_tensor("o", [S, EH], FP32, kind="ExternalOutput")

    def ring(i):
        return nc.sync if i % 2 == 0 else nc.scalar

    with tile.TileContext(nc) as tc:
        with (
            tc.tile_pool(name="xp", bufs=N_XT) as xp,
            tc.tile_pool(name="wp", bufs=4) as wp,
            tc.tile_pool(name="sp", bufs=1) as sp,
            tc.tile_pool(name="ps", bufs=1, space="PSUM") as ps,
        ):
            ones_col = sp.tile([P, 1], FP32, tag="ones_col")
            nc.vector.memset(ones_col[:], 1.0)
            ones_row = sp.tile([1, P], FP32, tag="ones_row")
            nc.vector.memset(ones_row[:], 1.0)

            xts = []
            for t in range(N_XT):
                xt = xp.tile([P, E], FP32, tag="x")
                ring(t).dma_start(xt[:], x_d[t * P : (t + 1) * P, :])
                xts.append(xt)

            bcr = sp.tile([1, EH], FP32, tag="bcr")
            nc.sync.dma_start(bcr[:], bc_d[None, :])
            wts = []
            for k in range(4):
                wt = wp.tile([P, EH], FP32, tag="w")
                ring(k).dma_start(wt[:], wc_d[k * P : (k + 1) * P, :])
                wts.append(wt)

            # serial accumulate: acc = sum_t x_t   [128, 512]
            acc = sp.tile([P, E], FP32, tag="acc")
            nc.vector.tensor_add(acc[:], xts[0][:], xts[1][:])
            for t in range(2, N_XT):
                nc.vector.tensor_add(acc[:], acc[:], xts[t][:])

            # column sums: xsum^T [128, 4]; xsumT[p, c] = sum_s x[s, c*128+p]
            p_red = ps.tile([P, 4], FP32, tag="red")
            for c in range(4):
                nc.tensor.matmul(
                    p_red[:, c : c + 1],
                    acc[:, c * P : (c + 1) * P],
                    ones_col[:],
                    start=True,
                    stop=True,
                )
            xsumT = sp.tile([P, 4], FP32, tag="xsumT")
            nc.vector.tensor_copy(xsumT[:], p_red[:])

            # c_row [1, 256] = xsum @ Wc_half  (accumulate over 4 k-chunks)
            p_crow = ps.tile([1, EH], FP32, tag="crow")
            for k in range(4):
                nc.tensor.matmul(
                    p_crow[:],
                    xsumT[:, k : k + 1],
                    wts[k][:],
                    start=(k == 0),
                    stop=(k == 3),
                )
            crow = sp.tile([1, EH], FP32, tag="crowsb")
            nc.vector.tensor_add(crow[:], p_crow[:], bcr[:])

            # broadcast row across 128 partitions via rank-1 matmul
            p_bc = ps.tile([P, EH], FP32, tag="bc")
            nc.tensor.matmul(p_bc[:], ones_row[:], crow[:], start=True, stop=True)
            bcast = sp.tile([P, EH], FP32, tag="bcast")
            nc.vector.tensor_copy(bcast[:], p_bc[:])

            for u in range(N_OT):
                ring(u).dma_start(o_d[u * P : (u + 1) * P, :], bcast[:])

    nc.compile()
    _CACHE["nc"] = nc
    return nc


def _fold_weights(qkv_w, qkv_b, out_w, out_b):
    wv = np.asarray(qkv_w)[:, 2 * E : 3 * E].astype(np.float64)
    wc = (wv @ np.asarray(out_w).astype(np.float64) / S).astype(np.float32)
    bc = (
        np.asarray(qkv_b)[2 * E : 3 * E].astype(np.float64)
        @ np.asarray(out_w).astype(np.float64)
        + np.asarray(out_b)
    ).astype(np.float32)
    return wc, bc


def _run(inputs, trace=False, **kwargs):
    nc = build()
    x = np.ascontiguousarray(np.asarray(inputs["x"], dtype=np.float32))
    wc, bc = _fold_weights(
        inputs["qkv_w"], inputs["qkv_b"], inputs["out_w"], inputs["out_b"]
    )
    in_maps = [
        {
            "x": x[c // 2],
            "wc": np.ascontiguousarray(wc[:, (c % 2) * EH : (c % 2 + 1) * EH]),
            "bc": np.ascontiguousarray(bc[(c % 2) * EH : (c % 2 + 1) * EH]),
        }
        for c in range(N_CORES)
    ]
    res = run_bass_kernel_spmd(
        nc, in_maps, core_ids=list(range(N_CORES)), trace=trace, **kwargs
    )
    out = np.empty((B, S, E), dtype=np.float32)
    for b in range(B):
        out[b, :, :EH] = res.results[2 * b]["o"]
        out[b, :, EH:] = res.results[2 * b + 1]["o"]
    return out, res


def kernel(**inputs) -> np.ndarray:
    out, _ = _run(inputs, trace=False)
    return out
